# revision 1
# baseline (speedup 1.0000x reference)
"""Trainium2 Bass kernel for CompressedLinear:
    y = x @ (int8_W * scale).T + fp16_bias
  x: (2, 2048, 4096) fp32, W: (16384, 4096) int8, scale: () fp32, bias: (16384,) fp16
  out: (2, 2048, 16384) fp32

Strategy (tensor parallel over out_features, 8 cores x 2048 outs):
  - int8 weights are EXACTLY representable in fp16 -> matmul in fp16 at full
    PE rate (1 cycle/row).  x is cast to fp16 on host (rel err ~2^-12).
  - Host pre-transposes both operands into k-major tiled layouts so every DMA
    is fully contiguous per partition and no on-chip transposes are needed:
      xt [ki=128, mo=32, ko=32, mi=128]  (shared by all cores)
      wt [ki=128, ko=32, n=2048]         (per-core shard, fp16 == exact int8)
  - Per core: wt resident in SBUF (16MB).  Loop 32 m-tiles: DMA x-tile,
    128 matmuls (psum[128m,512n] += xt[ko].T @ wt[ko, chunk]), evict via
    ScalarE activation Copy with scale (runtime value via [128,1] AP), add
    bias on VectorE from a host-broadcast [128,2048] tile, store y m-row.
"""

import os
import sys

import numpy as np

_TRN_REPO = "/opt/trn_rl_repo"
for _p in (_TRN_REPO, os.path.join(_TRN_REPO, "..")):
    if os.path.isdir(_TRN_REPO) and _p not in sys.path:
        sys.path.insert(0, _p)

import concourse.bass as bass  # noqa: E402
import concourse.mybir as mybir  # noqa: E402
import concourse.tile as tile  # noqa: E402
from concourse import bacc, bass_utils  # noqa: E402
from concourse.bass import ts  # noqa: E402

P = 128
N_CORES = 8


def build_module(m_tiles=32, k_tiles=32, n_shard=2048, n_free=512):
    """One NeuronCore's program; SPMD across cores with different wt/bias."""
    n_chunks = n_shard // n_free
    nc = bacc.Bacc("TRN2", target_bir_lowering=False, debug=False)

    xt = nc.dram_tensor(
        "xt", [P, m_tiles, k_tiles, P], mybir.dt.float16, kind="ExternalInput"
    )
    wt = nc.dram_tensor(
        "wt", [P, k_tiles, n_shard], mybir.dt.float16, kind="ExternalInput"
    )
    biasb = nc.dram_tensor(
        "biasb", [P, n_shard], mybir.dt.float32, kind="ExternalInput"
    )
    scalev = nc.dram_tensor("scalev", [P, 1], mybir.dt.float32, kind="ExternalInput")
    y = nc.dram_tensor(
        "y", [m_tiles * P, n_shard], mybir.dt.float32, kind="ExternalOutput"
    )
    yv = y[:].rearrange("(mo mi) n -> mi mo n", mi=P)

    with tile.TileContext(nc) as tc:
        with (
            tc.tile_pool(name="consts", bufs=1) as consts,
            tc.tile_pool(name="xp", bufs=3) as xp,
            tc.tile_pool(name="yp", bufs=2) as yp,
            tc.tile_pool(name="pp", bufs=8, space="PSUM") as pp,
        ):
            # PE warmup: ~24 dummy matmuls on memset scratch so the HAM clock
            # gate reaches 8/8 while the weight DMAs stream in.
            wu_lhs = consts.tile([P, P], mybir.dt.float16, name="wu_lhs")
            wu_rhs = consts.tile([P, n_free], mybir.dt.float16, name="wu_rhs")
            nc.any.memset(wu_lhs[:], 0.0)
            nc.any.memset(wu_rhs[:], 0.0)
            wu_ps = pp.tile([P, n_free], mybir.dt.float32, tag="ps", name="wu_ps")
            for _ in range(36):
                nc.tensor.matmul(wu_ps[:], wu_lhs[:], wu_rhs[:], start=True, stop=True)

            # First x-tiles on the Sync HWDGE ring, weights on the Scalar ring
            # (separate FIFOs), y-stores on GpSimd SWDGE.
            xt_tiles = {}

            def load_xt(mo):
                t = xp.tile(
                    [P, k_tiles, P], mybir.dt.float16, tag="xt_sb", name=f"xt_sb_{mo}"
                )
                nc.sync.dma_start(t[:], xt[:, mo])
                xt_tiles[mo] = t

            load_xt(0)
            load_xt(1)

            # Per-ko weight tiles -> fine-grained deps: matmuls for ko start
            # as soon as that 512KB slice lands, not after the full 16MB.
            scale_sb = consts.tile([P, 1], mybir.dt.float32, name="scale_sb")
            nc.scalar.dma_start(scale_sb[:], scalev[:])
            wt_sb = [
                consts.tile([P, n_shard], mybir.dt.float16, name=f"wt_sb_{ko}")
                for ko in range(k_tiles)
            ]
            for ko in range(k_tiles):
                nc.scalar.dma_start(wt_sb[ko][:], wt[:, ko])
            bias_sb = consts.tile([P, n_shard], mybir.dt.float32, name="bias_sb")
            nc.scalar.dma_start(bias_sb[:], biasb[:])

            for mo in range(m_tiles):
                if mo + 2 < m_tiles:
                    load_xt(mo + 2)
                xt_sb = xt_tiles.pop(mo)
                y_sb = yp.tile(
                    [P, n_shard], mybir.dt.float32, tag="y_sb", name=f"y_sb_{mo}"
                )
                psums = [
                    pp.tile([P, n_free], mybir.dt.float32, tag="ps", name=f"ps_{mo}_{c}")
                    for c in range(n_chunks)
                ]
                def evict(c):
                    # y = (psum * scale) + bias in one DVE op
                    nc.vector.scalar_tensor_tensor(
                        out=y_sb[:, ts(c, n_free)],
                        in0=psums[c][:],
                        scalar=scale_sb[:],
                        in1=bias_sb[:, ts(c, n_free)],
                        op0=mybir.AluOpType.mult,
                        op1=mybir.AluOpType.add,
                    )

                if mo < 2:
                    # ko-major: rides the incoming W stream k-tile by k-tile
                    for ko in range(k_tiles):
                        lhsT = xt_sb[:, ko]
                        for c in range(n_chunks):
                            nc.tensor.matmul(
                                psums[c][:],
                                lhsT,
                                wt_sb[ko][:, ts(c, n_free)],
                                start=(ko == 0),
                                stop=(ko == k_tiles - 1),
                            )
                    for c in range(n_chunks):
                        evict(c)
                    nc.scalar.dma_start(yv[:, mo], y_sb[:])
                else:
                    # chunk-major: each chunk finishes early -> eager evict
                    # + store, shortening the kernel tail
                    for c in range(n_chunks):
                        for ko in range(k_tiles):
                            nc.tensor.matmul(
                                psums[c][:],
                                xt_sb[:, ko],
                                wt_sb[ko][:, ts(c, n_free)],
                                start=(ko == 0),
                                stop=(ko == k_tiles - 1),
                            )
                        evict(c)
                        nc.scalar.dma_start(
                            yv[:, mo, ts(c, n_free)], y_sb[:, ts(c, n_free)]
                        )

    nc.compile()
    return nc


def prep_inputs(x, compressed_weight, scale, compressed_bias, n_cores=N_CORES):
    """Host-side shard + layout prep. Returns per-core in_maps."""
    x = np.asarray(x, dtype=np.float32)
    w = np.asarray(compressed_weight)
    bias = np.asarray(compressed_bias).astype(np.float32)
    scale_f = np.float32(scale)

    m_total, k_total = x.reshape(-1, x.shape[-1]).shape
    n_total = w.shape[0]
    m_tiles, k_tiles = m_total // P, k_total // P
    n_shard = n_total // n_cores

    x2 = x.reshape(m_total, k_total).astype(np.float16)
    # [mo, mi, ko, ki] -> [ki, mo, ko, mi]
    xt = np.ascontiguousarray(
        x2.reshape(m_tiles, P, k_tiles, P).transpose(3, 0, 2, 1)
    )
    scalev = np.full((P, 1), scale_f, dtype=np.float32)

    in_maps = []
    for s in range(n_cores):
        ws = w[s * n_shard : (s + 1) * n_shard].astype(np.float16)  # exact int8
        # [n, ko, ki] -> [ki, ko, n]
        wts = np.ascontiguousarray(ws.reshape(n_shard, k_tiles, P).transpose(2, 1, 0))
        bs = bias[s * n_shard : (s + 1) * n_shard]
        biasb = np.ascontiguousarray(np.broadcast_to(bs, (P, n_shard)))
        in_maps.append({"xt": xt, "wt": wts, "biasb": biasb, "scalev": scalev})
    return in_maps


_NC_CACHE = {}


def _get_module():
    key = "full"
    if key not in _NC_CACHE:
        _NC_CACHE[key] = build_module()
    return _NC_CACHE[key]


def run_on_hw(in_maps, **kwargs):
    nc = _get_module()
    return bass_utils.run_bass_kernel_spmd(
        nc, in_maps, core_ids=list(range(len(in_maps))), **kwargs
    )


def kernel(x, compressed_weight, scale, compressed_bias):
    in_maps = prep_inputs(x, compressed_weight, scale, compressed_bias)
    last_err = None
    for _attempt in range(3):  # rare transient NRT device errors
        try:
            res = run_on_hw(in_maps)
            break
        except Exception as e:  # noqa: BLE001
            last_err = e
    else:
        raise last_err
    shards = [np.asarray(res.results[i]["y"]) for i in range(N_CORES)]
    y = np.concatenate(shards, axis=1)
    return y.reshape(2, 2048, 16384)

